# revision 22
# baseline (speedup 1.0000x reference)
"""Trainium2 Bass kernel for nn_DataEmbedding_cycle_pos.

Math (B=16, T=2048, N=8, D=512), out[b,t,:] =
    conv(x)               Conv1d(N->D, k=3, circular)        -> matmul K=24
  + temporal(x_mark)      sum of 4 fixed-table lookups; all indices < 7 and
                          the 4 tables share rows 0..6, so it's
                          onehot28 @ R4 (R4 = tile(R7, 4))    -> matmul K=28
  + cycle-positional      periods = clip(T/freq[argmax |rfft|], 1, T); for
                          T=2048 the period is 2048 unless the argmax is
                          exactly the Nyquist bin (then 1.0).  Per (b,n) only
                          the bit "is Nyquist the strict max" matters:
                            cyc[b] = (1-cnt/8)*postab + (cnt/8)*row01
                          cnt = #Nyquist-max series in batch b.
  The row01 (odd-column ones) term folds into the onehot matmul rows since
  sum(onehot) == 4 exactly:  R4 + (cnt/32)*odd.  The postab term is applied
  per tile by one fused DVE op: out = (postab_tile * a_vec) + psum.

Sharding: batch-parallel (2 batches/core).  The |rfft|^2 argmax test is
computed per core for its OWN 16 series (no collectives) via a
quarter-size DFT: double time-fold (u = 0..512 with boundary-corrected
butterfly operands) x frequency-parity split, all in bf16 matmuls.
Input DMAs ride the Scalar HWDGE ring; output DMAs ride the Sync ring.
"""
import sys, os

sys.path.insert(0, "/opt/trn_rl_repo")
import numpy as np
import ml_dtypes

import concourse.bass as bass
import concourse.bacc as bacc
import concourse.mybir as mybir
import concourse.tile as tile
from concourse.bass_utils import run_bass_kernel_spmd

B, T, N, D = 16, 2048, 8, 512
NCORES = 8
BPC = B // NCORES          # batches per core
SPC = BPC * N              # series per core (16)
NT = T // 128              # 128-row time tiles per batch
KCONV = 3 * N              # 24 conv rows
KHOT = 28                  # 4 features x 7 index values
KTOT = KCONV + KHOT        # 52
NYQ = T // 2               # 1024
UCH = 5                    # u chunks of 128 covering u=0..512
FEW = 516                  # even-parity freq cols (Nyquist first, 3 pad)

F32 = mybir.dt.float32
BF16 = mybir.dt.bfloat16
F32R = mybir.dt.float32r
BF = ml_dtypes.bfloat16

TRACE = False
TRACE_DIR = None

_cache = {}


# ----------------------------------------------------------------- constants
def _div_term():
    # mirror reference: exp(arange(0,512,2) * (-ln 10000 / 512)) in f32
    return np.exp(
        np.arange(0, D, 2, dtype=np.float32) * np.float32(-np.log(10000.0) / D)
    ).astype(np.float32)


def _fixed_rows(nrows):
    pos = np.arange(nrows, dtype=np.float32)[:, None]
    ang = (pos * _div_term()[None, :]).astype(np.float32)
    tab = np.zeros((nrows, D), dtype=np.float32)
    tab[:, 0::2] = np.sin(ang)
    tab[:, 1::2] = np.cos(ang)
    return tab


def _host_constants():
    c = {}
    postab = _fixed_rows(T)  # [2048, 512]
    # SBUF layout [128(tt), 16tiles * 512]
    c["postab"] = np.ascontiguousarray(
        postab.reshape(NT, 128, D).transpose(1, 0, 2).reshape(128, NT * D)
    ).astype(BF)
    r7 = _fixed_rows(7)
    odd = np.zeros((D,), dtype=np.float32)
    odd[1::2] = 1.0
    c["odd28"] = np.tile(odd[None, :], (KHOT, 1)).astype(np.float32)
    c["v28"] = np.tile(np.arange(7, dtype=np.float32), 4)[:, None].copy()
    c["r4"] = np.ascontiguousarray(np.tile(r7, (4, 1))).astype(np.float32)

    # quarter DFT tables (double time-fold, frequencies split by parity):
    # operands are 4 boundary-corrected butterflies of x over u=0..512 and
    # each table is [u-rows, parity-freq-cols].  Layout [128(tt), 5 ch * w].
    uu = np.arange(UCH * 128, dtype=np.float64)                # u padded
    umask = (uu <= 512).astype(np.float64)
    w = 2.0 * np.pi / T
    ke = np.arange(0, NYQ + 1, 2, dtype=np.float64)            # 513 even
    ko = np.arange(1, NYQ, 2, dtype=np.float64)                # 512 odd
    kep = np.concatenate([ke[512:], ke[:512]])                 # nyq first
    # even tables padded to 516 cols: col 0 = k=1024 (Nyquist), 1.. = rest
    kev = np.zeros(FEW, dtype=np.float64); kev[:513] = kep
    kevm = np.zeros(FEW); kevm[:513] = 1.0

    def _tab(fn, kcols, kmask_):
        m = (fn(w * np.outer(uu, kcols)) * kmask_[None, :] * umask[:, None])
        m = m.astype(np.float32)
        ncol = len(kcols)
        return np.ascontiguousarray(
            m.reshape(UCH, 128, ncol).transpose(1, 0, 2).reshape(128, UCH * ncol)
        ).astype(BF)

    c["cose"] = _tab(np.cos, kev, kevm)
    c["sine"] = _tab(np.sin, kev, kevm)
    c["coso"] = _tab(np.cos, ko, np.ones(512))
    c["sino"] = _tab(np.sin, ko, np.ones(512))

    # batch indicator for the cnt matmul: ind2[s, i*128+p] = (s//8 == i)
    s_batch = np.arange(SPC) // N
    cols = [np.tile((s_batch == i).astype(np.float32)[:, None], (1, 128))
            for i in range(BPC)]
    c["ind2"] = np.concatenate(cols, axis=1).astype(BF)        # [16, 256]
    return c


# ------------------------------------------------------------------- program
def _build_nc():
    nc = bacc.Bacc("TRN2", target_bir_lowering=False, debug=False,
                   num_devices=NCORES)

    def din(name, shape, dt):
        return nc.dram_tensor(name, shape, dt, kind="ExternalInput").ap()

    xtp = din("xtp", [BPC, N, T + 2], F32R)       # circular-padded x^T
    xm7 = din("xm7", [BPC, KHOT, T], F32)         # x_mark rows repeated 7x
    xa = din("xa", [128, UCH * SPC], BF16)        # x[u]
    xb = din("xb", [128, UCH * SPC], BF16)        # x[2048-u] (u=0 ->0, 512->x1536)
    xc = din("xc", [128, UCH * SPC], BF16)        # x[1024-u] (u=0 -> x1024)
    xd = din("xd", [128, UCH * SPC], BF16)        # x[1024+u]
    cose = din("cose", [128, UCH * FEW], BF16)
    sine = din("sine", [128, UCH * FEW], BF16)
    coso = din("coso", [128, UCH * 512], BF16)
    sino = din("sino", [128, UCH * 512], BF16)
    postab = din("postab", [128, NT * D], BF16)
    w24 = din("w24", [KCONV, D], F32R)
    r4 = din("r4", [KHOT, D], F32)
    odd28 = din("odd28", [KHOT, D], F32)
    v28 = din("v28", [KHOT, 1], F32)
    ind2 = din("ind2", [SPC, BPC * 128], BF16)
    out = nc.dram_tensor("out", [BPC, T, D], F32, kind="ExternalOutput").ap()

    with tile.TileContext(nc) as tc:
        with (
            tc.tile_pool(name="consts", bufs=1) as cpool,
            tc.tile_pool(name="fwork", bufs=1) as fpool,
            tc.tile_pool(name="fpsum", bufs=1, space="PSUM") as fpsum,
            tc.tile_pool(name="cpsum", bufs=1, space="PSUM") as cpsum,
            tc.tile_pool(name="mpsum", bufs=5, space="PSUM") as mpsum,
            tc.tile_pool(name="batch", bufs=2) as bpool,
            tc.tile_pool(name="outp", bufs=6) as opool,
        ):
            # ---------------- FFT phase: own-series |rfft|^2 over all bins
            # butterfly operands: 4 DMAs + 8 tiny DVE combines
            xa_sb = fpool.tile([128, UCH * SPC], BF16, tag="xa")
            nc.scalar.dma_start(xa_sb[:], xa)
            xb_sb = fpool.tile([128, UCH * SPC], BF16, tag="xb")
            nc.scalar.dma_start(xb_sb[:], xb)
            xc_sb = fpool.tile([128, UCH * SPC], BF16, tag="xc")
            nc.scalar.dma_start(xc_sb[:], xc)
            xd_sb = fpool.tile([128, UCH * SPC], BF16, tag="xd")
            nc.scalar.dma_start(xd_sb[:], xd)
            ab = fpool.tile([128, UCH * SPC], BF16, tag="ab")
            nc.vector.tensor_add(ab[:], xa_sb[:], xb_sb[:])
            amb = fpool.tile([128, UCH * SPC], BF16, tag="amb")
            nc.vector.tensor_sub(amb[:], xa_sb[:], xb_sb[:])
            cd = fpool.tile([128, UCH * SPC], BF16, tag="cd")
            nc.vector.tensor_add(cd[:], xc_sb[:], xd_sb[:])
            cmd = fpool.tile([128, UCH * SPC], BF16, tag="cmd")
            nc.vector.tensor_sub(cmd[:], xc_sb[:], xd_sb[:])
            pce = fpool.tile([128, UCH * SPC], BF16, tag="pce")
            nc.vector.tensor_add(pce[:], ab[:], cd[:])
            pco = fpool.tile([128, UCH * SPC], BF16, tag="pco")
            nc.vector.tensor_sub(pco[:], ab[:], cd[:])
            pse = fpool.tile([128, UCH * SPC], BF16, tag="pse")
            nc.vector.tensor_sub(pse[:], amb[:], cmd[:])
            pso = fpool.tile([128, UCH * SPC], BF16, tag="pso")
            nc.vector.tensor_add(pso[:], amb[:], cmd[:])

            # per-chunk table tiles so matmuls start as chunks land
            tabs = {}
            for nm, dram, ncol in (("ce", cose, FEW), ("se", sine, FEW),
                                   ("co", coso, 512), ("so", sino, 512)):
                for ch in range(UCH):
                    t_ = cpool.tile([128, ncol], BF16, tag=f"{nm}{ch}")
                    nc.scalar.dma_start(t_[:], dram[:, ch * ncol:(ch + 1) * ncol])
                    tabs[(nm, ch)] = t_

            # psum bank A: re_e/re_o/im_e/im_o [16, 512] at bases 0/32/64/96
            # (even tables: col 0 is Nyquist -> lands in ps_tail)
            ps_main = fpsum.tile([112, 512], F32, tag="psmain")
            ps_tail = fpsum.tile([48, 4], F32, tag="pstail")
            opnds = {"ce": pce, "co": pco, "se": pse, "so": pso}
            for ch in range(UCH):
                st = (ch == 0); sp = (ch == UCH - 1)
                for nm, tb in (("ce", 0), ("se", 32)):
                    nc.tensor.matmul(ps_tail[tb:tb + SPC, :],
                                     opnds[nm][:, ch * SPC:(ch + 1) * SPC],
                                     tabs[(nm, ch)][:, 0:4],
                                     start=st, stop=sp,
                                     tile_position=(0, tb))
                for base, nm in ((0, "ce"), (32, "co"), (64, "se"), (96, "so")):
                    lhs = opnds[nm][:, ch * SPC:(ch + 1) * SPC]
                    cols = tabs[(nm, ch)][:, 4:FEW] if nm in ("ce", "se") \
                        else tabs[(nm, ch)][:, 0:512]
                    nc.tensor.matmul(ps_main[base:base + SPC, :], lhs, cols,
                                     start=st, stop=sp,
                                     tile_position=(0, base))

            mag_e = fpool.tile([SPC, 512], F32, tag="mag_e")
            mag_o = fpool.tile([SPC, 512], F32, tag="mag_o")
            mag_t = fpool.tile([SPC, 4], F32, tag="mag_t")
            sqa = fpool.tile([SPC, 512], F32, tag="sqa")
            sqb = fpool.tile([SPC, 512], F32, tag="sqb")
            sqc = fpool.tile([SPC, 4], F32, tag="sqc")
            nc.scalar.square(mag_e[:], ps_main[0:SPC, :])
            nc.scalar.square(sqa[:], ps_main[64:64 + SPC, :])
            nc.gpsimd.tensor_add(mag_e[:], mag_e[:], sqa[:])
            nc.scalar.square(mag_o[:], ps_main[32:32 + SPC, :])
            nc.scalar.square(sqb[:], ps_main[96:96 + SPC, :])
            nc.gpsimd.tensor_add(mag_o[:], mag_o[:], sqb[:])
            nc.scalar.square(mag_t[:], ps_tail[0:SPC, :])
            nc.scalar.square(sqc[:], ps_tail[32:32 + SPC, :])
            nc.gpsimd.tensor_add(mag_t[:], mag_t[:], sqc[:])

            # strict >: Nyquist wins only if greater than every earlier bin
            # (main cols cover k>=6; tail cols 1:4 cover k=0,2,4)
            lm_e = fpool.tile([SPC, 1], F32, tag="lm_e")
            nc.vector.reduce_max(lm_e[:], mag_e[:], axis=mybir.AxisListType.X)
            lm_o = fpool.tile([SPC, 1], F32, tag="lm_o")
            nc.vector.reduce_max(lm_o[:], mag_o[:], axis=mybir.AxisListType.X)
            lm_t = fpool.tile([SPC, 1], F32, tag="lm_t")
            nc.vector.reduce_max(lm_t[:], mag_t[:, 1:4],
                                 axis=mybir.AxisListType.X)
            lm2 = fpool.tile([SPC, 1], F32, tag="lm2")
            nc.vector.tensor_max(lm2[:], lm_e[:], lm_o[:])
            lmax = fpool.tile([SPC, 1], F32, tag="lmax")
            nc.vector.tensor_max(lmax[:], lm2[:], lm_t[:])
            isn = fpool.tile([SPC, 1], BF16, tag="isn")
            nc.vector.tensor_tensor(isn[:], mag_t[:, 0:1], lmax[:],
                                    op=mybir.AluOpType.is_gt)

            ind2_sb = cpool.tile([SPC, BPC * 128], BF16, tag="ind2")
            nc.scalar.dma_start(ind2_sb[:], ind2)

            a_vecs, bq_vecs = [], []
            for i in range(BPC):
                ps_cnt = cpsum.tile([128, 1], F32, tag="pscnt")
                nc.tensor.matmul(ps_cnt[:], ind2_sb[:, i * 128:(i + 1) * 128],
                                 isn[:], start=True, stop=True)
                a_vec = fpool.tile([128, 1], F32, tag=f"avec{i}")
                nc.vector.tensor_scalar(a_vec[:], ps_cnt[:], -0.125, 1.0,
                                        op0=mybir.AluOpType.mult,
                                        op1=mybir.AluOpType.add)
                bq_vec = fpool.tile([128, 1], F32, tag=f"bqvec{i}")
                nc.vector.tensor_scalar(bq_vec[:], ps_cnt[:], 1.0 / 32.0,
                                        None, op0=mybir.AluOpType.mult)
                a_vecs.append(a_vec)
                bq_vecs.append(bq_vec)

            # ---------------- constants for the main matmul
            postab_sb = cpool.tile([128, NT * D], BF16, tag="postab")
            nc.scalar.dma_start(postab_sb[:], postab)
            r4_sb = cpool.tile([KHOT, D], F32, tag="r4")
            nc.scalar.dma_start(r4_sb[:], r4)
            odd28_sb = cpool.tile([KHOT, D], F32, tag="odd28")
            nc.scalar.dma_start(odd28_sb[:], odd28)
            v28_sb = cpool.tile([KHOT, 1], F32, tag="v28")
            nc.scalar.dma_start(v28_sb[:], v28)

            # ---------------- main per-batch pipelines
            # lt row layout: [0:28] onehot (ACT-written, base partition 0),
            #                [28:52] conv x rows (DMA-written, any base legal)
            for i in range(BPC):
                lt = bpool.tile([KTOT, T], F32R, tag="lt")
                for k in range(3):
                    nc.scalar.dma_start(lt[KHOT + k * N:KHOT + (k + 1) * N, :],
                                        xtp[i, :, k:k + T])
                xm = bpool.tile([KHOT, T], F32, tag="xm")
                nc.scalar.dma_start(xm[:], xm7[i])
                t28 = bpool.tile([KHOT, T], F32, tag="t28")
                # t28 = |xm - v|;  lt[0:28] = relu(1 - t28) = onehot
                nc.scalar.activation(t28[:], xm[:],
                                     mybir.ActivationFunctionType.Abs,
                                     bias=v28_sb[:], scale=-1.0)
                nc.scalar.activation(lt[0:KHOT, :], t28[:],
                                     mybir.ActivationFunctionType.Relu,
                                     bias=1.0, scale=-1.0)

                rhs = bpool.tile([KTOT, D], F32R, tag="rhs")
                nc.scalar.dma_start(rhs[KHOT:KTOT, :], w24)
                # R4 + (cnt/32)*odd: sum(onehot)==4 folds the odd term
                nc.vector.scalar_tensor_tensor(
                    rhs[0:KHOT, :], odd28_sb[:], bq_vecs[i][0:KHOT, :],
                    r4_sb[:], op0=mybir.AluOpType.mult, op1=mybir.AluOpType.add)

                for ti in range(NT):
                    ps = mpsum.tile([128, D], F32, tag="ps")
                    nc.tensor.matmul(ps[:],
                                     lt[:, ti * 128:(ti + 1) * 128],
                                     rhs[:],
                                     start=True, stop=True)
                    ot = opool.tile([128, D], F32, tag="ot")
                    nc.vector.scalar_tensor_tensor(
                        ot[:], postab_sb[:, ti * D:(ti + 1) * D], a_vecs[i][:],
                        ps[:], op0=mybir.AluOpType.mult, op1=mybir.AluOpType.add)
                    nc.sync.dma_start(out[i, ti * 128:(ti + 1) * 128, :], ot[:])
    nc.compile()
    return nc


def _get_nc():
    if "nc" not in _cache:
        _cache["nc"] = _build_nc()
    return _cache["nc"]


def _host_inputs(x, x_mark, conv_w):
    # x^T with circular pad: xtp[b, n, j] = x[b, (j-1) % T, n]
    xt = np.ascontiguousarray(x.transpose(0, 2, 1))        # [16, 8, 2048]
    xtp = np.concatenate([xt[:, :, -1:], xt, xt[:, :, :1]], axis=2)
    # x_mark as f32, transposed, each feature row repeated 7x -> [16, 28, T]
    xmt = x_mark.astype(np.float32).transpose(0, 2, 1)     # [16, 4, 2048]
    xm7 = np.repeat(xmt, 7, axis=1)                        # [16, 28, 2048]
    # per-core butterfly operands [tt, ch*16 + s], u = ch*128+tt (0..512)
    uu = np.arange(UCH * 128)
    val = uu <= 512
    mid = (uu >= 1) & (uu <= 511)
    ai = np.where(val, np.minimum(uu, 512), 0)
    bi = np.where(mid, (T - uu) % T, np.where(uu == 512, 1536, 0))
    bm = val.copy()
    ci = np.where(mid, NYQ - uu, np.where(uu == 0, NYQ, 0))
    cm = mid | (uu == 0)
    di = np.where(mid, NYQ + uu, 0)
    dm = mid
    quads = []
    for core in range(NCORES):
        xs = x[core * BPC:(core + 1) * BPC]                # [2, 2048, 8]
        xflat = xs.transpose(1, 0, 2).reshape(T, SPC)      # [t, s]
        qs = []
        for idx, msk in ((ai, val), (bi, bm), (ci, cm), (di, dm)):
            arr = xflat[idx] * msk[:, None]
            qs.append(np.ascontiguousarray(
                arr.reshape(UCH, 128, SPC).transpose(1, 0, 2)
                   .reshape(128, UCH * SPC)).astype(BF))
        quads.append(qs)
    # conv weight rows (k, n): w24[k*8+n, d] = conv_w[d, n, k]
    w24 = np.ascontiguousarray(conv_w.transpose(2, 1, 0).reshape(KCONV, D))
    return xtp, xm7, quads, w24


def make_in_maps(x, x_mark, conv_w):
    if "consts" not in _cache:
        _cache["consts"] = _host_constants()
    c = _cache["consts"]
    xtp, xm7, quads, w24 = _host_inputs(x, x_mark, conv_w)
    in_maps = []
    for core in range(NCORES):
        b0 = core * BPC
        in_maps.append({
            "xtp": np.ascontiguousarray(xtp[b0:b0 + BPC]),
            "xm7": np.ascontiguousarray(xm7[b0:b0 + BPC]),
            "xa": quads[core][0],
            "xb": quads[core][1],
            "xc": quads[core][2],
            "xd": quads[core][3],
            "cose": c["cose"],
            "sine": c["sine"],
            "coso": c["coso"],
            "sino": c["sino"],
            "postab": c["postab"],
            "w24": w24.astype(np.float32),
            "r4": c["r4"],
            "odd28": c["odd28"],
            "v28": c["v28"],
            "ind2": c["ind2"],
        })
    return in_maps


# -------------------------------------------------------------------- driver
def kernel(**inputs):
    x = np.asarray(inputs["x"], dtype=np.float32)          # [16, 2048, 8]
    x_mark = np.asarray(inputs["x_mark"])                  # [16, 2048, 4] int
    conv_w = np.asarray(inputs["conv_w"], dtype=np.float32)  # [512, 8, 3]

    in_maps = make_in_maps(x, x_mark, conv_w)
    nc = _get_nc()
    kw = {}
    if TRACE:
        kw = dict(trace=True, tmpdir=TRACE_DIR)
    br = run_bass_kernel_spmd(nc, in_maps, list(range(NCORES)), **kw)
    if TRACE:
        _cache["last_results"] = br

    outp = np.empty((B, T, D), dtype=np.float32)
    for core in range(NCORES):
        outp[core * BPC:(core + 1) * BPC] = br.results[core]["out"]
    return outp
